# revision 25
# baseline (speedup 1.0000x reference)
"""CrossNet forward as a Trainium2 Bass/Tile kernel, data-parallel over 8 cores.

Math: the CrossNet layer stack
    x_{l+1} = x0 * (x_l . w_l) + b_l + x_l            (l = 0..3)
collapses in closed form.  Writing x_l = x0 * alpha_l[b] + beta_l[d]:
    p_l[b]     = sum_d x0[b,d] w_l[d]                 (4 projections of x0)
    alpha_0    = 1,   alpha_{l+1} = alpha_l * (1 + p_l) + c_l
    beta_{l+1} = beta_l + b_l,  c_l = beta_l . w_l    (host-computable scalars)
    out        = x0 * alpha_4[b] + beta_4[d]

Memory-bound: 16 MB fp16 in + 16 MB fp16 out per core.  Pure-DMA floor
measured at 87.8 us (16 DMA engines, ~433 GB/s/core with 8 cores active).

v8 design (d-major layout):
- Host packs x fp16 D-MAJOR: xd[d, r] = x[r, d], so projections are direct
  PE matmuls (lhsT = x chunk [128d,128b], rhs = W [128d,4]) with NO
  transposes and NO PSUM->SBUF staging copies.  The previous row-major
  design spent 1.29 us/supertile on TensorE (4 packed f32 transposes + 8
  projections) + 0.65 us on ACT copies, which exceeded the 1.21
  us/supertile DMA pace and made TensorE the critical path (measured
  99.5 us).  d-major cuts TensorE to ~0.9 us/supertile.
- alpha comes out row-indexed [128b, G]; the final multiply in d-major
  needs it broadcast along the free dim.  Chain per PAIR of supertiles:
  PE transpose a16 [128,16]->[16,128] PSUM, ACT copy to SBUF, SBUF->SBUF
  flatten DMA [16,128]->[1,2048] (APs linearize independently), then per
  supertile two K=1 ones-matmuls [1,512]->[128,512] rebroadcast alpha
  into PSUM, one ACT copy -> fp16 [128,1024], one DVE 2x_1P multiply.
- Stores alternate between the gpsimd SWDGE ring and the scalar HWDGE
  ring: one ring's per-transfer boundary overhead made DMA engines 0/15
  ~10% hotter than the rest and their queue drained ~6 us after all other
  engines finished (measured).  Head/tail stores are single-supertile to
  prime the store stream early and shorten the last final->store chains.
- Emission is software-pipelined with >= 1 supertile-tick of lag between
  dependent stages so no engine stream head-of-line blocks on a same-tick
  cross-engine dependency.
- 36-row remainder keeps the old row-major mini-chain (fp16 transpose via
  ident), injected mid-stream.  float32r transposes corrupt packed fp16
  on HW (measured rel err ~3.5) -- not used anywhere anymore.
"""

import numpy as np

B = 500_000
D = 128
L = 4
N_CORES = 8
ROWS = B // N_CORES          # 62500 rows per core
G = 8                        # 128-row chunks per supertile
SUP = 128 * G                # 1024 rows per supertile
NSUP = ROWS // SUP           # 61 full supertiles
REM = ROWS - NSUP * SUP      # 36 remainder rows
WPS = SUP                    # fp16 words per partition per supertile (d-major)

# Load group sizes: small groups first for fast pipeline ramp, then 1 MB
# transfers (8 KB per partition contiguous) for line-rate descriptors.
# (2-supertile groups were tried: compute stalls shrank but 4 KB
# descriptors inflated load DMA engine-time by 13% -- net wash.)
GROUPS = [1, 1, 2] + [4] * 13 + [2, 2, 1]
assert sum(GROUPS) == NSUP

_CACHE: dict = {}

# test.py can read run metadata (exec_time_ns etc.) from here after a call.
LAST_RESULTS = None


def _build(cs, has_bias):
    import concourse.tile as tile
    from concourse import bacc, mybir

    f32 = mybir.dt.float32
    f16 = mybir.dt.float16

    nc = bacc.Bacc(
        "TRN2",
        target_bir_lowering=False,
        debug=False,
        enable_asserts=False,
        num_devices=N_CORES,
    )
    # D-major main block: xd[d, s*1024 + r] = x(s*1024 + r, d)
    xd = nc.dram_tensor("xd", [128, NSUP * WPS], f16, kind="ExternalInput").ap()
    xrem = None
    if REM:
        xrem = nc.dram_tensor("xrem", [REM, D], f16, kind="ExternalInput").ap()
    w = nc.dram_tensor("w", [D, L], f16, kind="ExternalInput").ap()
    ident = nc.dram_tensor("ident", [128, 128], f16, kind="ExternalInput").ap()
    ones = nc.dram_tensor("ones", [1, 128], f16, kind="ExternalInput").ap()
    bb = None
    if has_bias:
        # d-major bias: beta[d] is a per-partition scalar
        bb = nc.dram_tensor("bb", [128, 1], f16, kind="ExternalInput").ap()
    # Output in the SAME d-major layout; host transposes back.
    out = nc.dram_tensor("out", [128, NSUP * WPS], f16, kind="ExternalOutput").ap()
    outr = None
    if REM:
        outr = nc.dram_tensor("outr", [REM, D], f16, kind="ExternalOutput").ap()

    with tile.TileContext(nc) as tc:
        with (
            tc.tile_pool(name="consts", bufs=1) as cpool,
            tc.tile_pool(name="xin", bufs=6) as xpool,
            tc.tile_pool(name="ptps", bufs=2, space="PSUM") as pps_pool,
            tc.tile_pool(name="atps", bufs=2, space="PSUM") as at_pool,
            tc.tile_pool(name="bcps", bufs=2, space="PSUM") as bc_pool,
            tc.tile_pool(name="small", bufs=16) as spool,
            tc.tile_pool(name="bcsb", bufs=4) as bcs_pool,
            tc.tile_pool(name="outp", bufs=8) as opool,
        ):
            # Constants ride the ACT HWDGE ring so xd loads own the SP ring.
            w_sb = cpool.tile([D, L], f16, tag="w")
            nc.scalar.dma_start(w_sb[:], w)
            ones_sb = cpool.tile([1, 128], f16, tag="ones")
            nc.scalar.dma_start(ones_sb[:], ones)
            ident_sb = cpool.tile([128, 128], f16, tag="ident")
            nc.scalar.dma_start(ident_sb[:], ident)
            bb_sb = None
            if has_bias:
                bb_sb = cpool.tile([128, 1], f16, tag="bb")
                nc.scalar.dma_start(bb_sb[:], bb)

            copy_fn = mybir.ActivationFunctionType.Copy

            def alpha_from_pt(pt_ap, p_cnt, g_cnt):
                # q = 1 + p on the ACT engine (reads PSUM, frees DVE cycles),
                # then product over the 4 layers; fp16 result.
                # pt_ap is an already-sliced AP of shape [p_cnt, L * g_cnt].
                q_sb = spool.tile([p_cnt, L * g_cnt], f32, tag="q")
                nc.scalar.activation(q_sb[:], pt_ap, copy_fn, bias=1.0)
                a16 = spool.tile([p_cnt, g_cnt], f16, tag="a16")
                if has_bias:
                    qv = q_sb[:].rearrange("p (g l) -> p g l", l=L)
                    a = spool.tile([p_cnt, g_cnt], f32, tag="a0")
                    # c_0 == 0 always (beta_0 = 0)
                    nc.vector.tensor_copy(a[:], qv[:, :, 0])
                    for l in range(1, L):
                        t = spool.tile([p_cnt, g_cnt], f32, tag=f"a{l}")
                        nc.vector.tensor_mul(t[:], a[:], qv[:, :, l])
                        if cs[l] != 0.0:
                            t2 = spool.tile([p_cnt, g_cnt], f32, tag=f"ac{l}")
                            nc.vector.tensor_scalar_add(t2[:], t[:], float(cs[l]))
                            t = t2
                        a = t
                    nc.vector.tensor_copy(a16[:], a[:])
                else:
                    # t[g, u] = q[g, 2u] * q[g, 2u+1], then a = t[:,0]*t[:,1]
                    qp = q_sb[:].rearrange("p (g u l) -> p g u l", u=2, l=2)
                    t = spool.tile([p_cnt, 2 * g_cnt], f32, tag="a1")
                    tv = t[:].rearrange("p (g u) -> p g u", u=2)
                    nc.vector.tensor_mul(tv, qp[:, :, :, 0], qp[:, :, :, 1])
                    nc.vector.tensor_mul(a16[:], tv[:, :, 0], tv[:, :, 1])
                return a16

            def stage_P(xv, pt_slice):
                # 8 projection matmuls straight off the d-major x tile.
                for g in range(G):
                    nc.tensor.matmul(
                        pt_slice[:, g * L : (g + 1) * L],
                        lhsT=xv[:, g * 128 : (g + 1) * 128],
                        rhs=w_sb[:],
                        start=True,
                        stop=True,
                    )

            NB = 2  # supertiles per store batch (512 KB stores)
            N_EARLY = 4
            pair_tile = [None]
            batch_state = [None]
            store_idx = [0]

            def emit_final(s, xv, bc_sb):
                # d-major final multiply: all operands [128, 1024] fp16
                # unit-stride SBUF -> DVE 2x_1P.
                if pair_tile[0] is None:
                    bsz = 1 if (s < N_EARLY or s >= NSUP - 4) else NB
                    batch_state[0] = [s, bsz]
                start_s, bsz = batch_state[0]
                half = s - start_s
                if half == 0:
                    pair_tile[0] = opool.tile(
                        [128, bsz * WPS], f16, name="opair", tag="o"
                    )
                out_sb = pair_tile[0]
                oslice = out_sb[:, half * WPS : (half + 1) * WPS]
                if has_bias:
                    t_sb = opool.tile([128, WPS], f16, tag="t")
                    nc.vector.tensor_mul(t_sb[:], xv, bc_sb[:])
                    nc.vector.tensor_add(
                        oslice.unsqueeze(1),
                        t_sb[:].unsqueeze(1),
                        bb_sb[:].to_broadcast([128, 1, WPS]),
                    )
                else:
                    nc.vector.tensor_mul(oslice, xv, bc_sb[:])
                if half == bsz - 1 or s == NSUP - 1:
                    n = (half + 1) * WPS
                    s0 = s - half
                    # Alternate stores across the gpsimd SWDGE ring and the
                    # scalar HWDGE ring (idle after consts).
                    store_idx[0] += 1
                    eng = nc.gpsimd if store_idx[0] % 2 == 0 else nc.scalar
                    eng.dma_start(
                        out[:, s0 * WPS : s0 * WPS + n], out_sb[:, :n]
                    )
                    pair_tile[0] = None

            def block_rem():
                # Row-major mini-chain for the 36 leftover rows.
                p_cnt = REM
                x_sb = spool.tile([p_cnt, D], f16, tag="xr")
                nc.scalar.dma_start(x_sb[:], xrem)
                xt_ps = at_pool.tile([128, p_cnt], f16, tag="at")
                xt_sb = spool.tile([128, p_cnt], f16, tag="xtsb")
                pt_ps = pps_pool.tile([p_cnt, L], f32, tag="pt")
                nc.tensor.transpose(xt_ps[:], x_sb[:], ident_sb[:p_cnt, :p_cnt])
                nc.scalar.copy(xt_sb[:], xt_ps[:])
                nc.tensor.matmul(
                    pt_ps[:], lhsT=xt_sb[:], rhs=w_sb[:], start=True, stop=True
                )
                a16 = alpha_from_pt(pt_ps[:], p_cnt, 1)
                out_sb = spool.tile([p_cnt, D], f16, tag="or")
                if has_bias:
                    t_sb = spool.tile([p_cnt, D], f16, tag="tr")
                    nc.vector.tensor_mul(
                        t_sb[:].rearrange("p (u d) -> p u d", u=1),
                        x_sb[:].rearrange("p (u d) -> p u d", u=1),
                        a16[:].to_broadcast([p_cnt, 1, D]),
                    )
                    # row-major: beta indexed along the free dim; build from
                    # the d-major bb via broadcast is not available here, so
                    # reuse the per-partition ident trick is overkill --
                    # instead add via a [1, D] view is not expressible; fall
                    # back to a second DRAM const would be needed.  Biases
                    # are zero for this problem; keep the mul-only path
                    # correct and add bias on the host for the remainder.
                    nc.vector.tensor_copy(out_sb[:], t_sb[:])
                else:
                    nc.vector.tensor_mul(
                        out_sb[:].rearrange("p (u d) -> p u d", u=1),
                        x_sb[:].rearrange("p (u d) -> p u d", u=1),
                        a16[:].to_broadcast([p_cnt, 1, D]),
                    )
                nc.gpsimd.dma_start(outr, out_sb[:])

            # Software-pipelined emission: every stage consumes work emitted
            # on an EARLIER tick so no engine stream stalls on a same-tick
            # cross-engine dependency.  Per pair of supertiles:
            #   t:   projections (Tensor; 2nd member at t)
            #   t+1: q 1+p (ACT) + alpha products (DVE)
            #   t+2: alphaT transpose [128,16]->[16,128] (Tensor)
            #   t+3: ACT copy -> SBUF fp16
            #   t+4: flatten DMA [16,128]->[1,2048] (gpsimd ring)
            #   t+6: 4x K=1 ones-matmul rebroadcast (Tensor; 2 ticks after
            #        the flatten to cover SWDGE latency)
            #   t+7: ACT copy PSUM->SBUF fp16 [128,1024] x2
            #   t+8: DVE finals + batched stores (via flights, lag >= 1)
            pair = []     # [(s, xv)] projected into pt_cur, awaiting pair
            pt_cur = [None]
            ready = []    # (tick, plist, pt) alpha pending
            s_alpha = []  # (tick, plist, a16) transpose pending
            s_tran = []   # (tick, plist, at_ps) copy pending
            s_copy = []   # (tick, plist, at_sb) flatten pending
            s_flat = []   # (tick, plist, aflat) bcast pending
            s_bc = []     # (tick, plist, [bc_ps x n]) bc-copy pending
            flights = []  # (s, xv, bc_sb) final pending

            def do_alpha(tick):
                t0, plist, pt = ready.pop(0)
                n = len(plist)
                a16 = alpha_from_pt(pt[:, : n * L * G], 128, n * G)
                s_alpha.append((tick, plist, a16))

            def do_tran(tick):
                t0, plist, a16 = s_alpha.pop(0)
                n = len(plist)
                at_ps = at_pool.tile([n * G, 128], f16, tag="at")
                nc.tensor.transpose(at_ps[:], a16[:], ident_sb[:])
                s_tran.append((tick, plist, at_ps))

            def do_copy(tick):
                t0, plist, at_ps = s_tran.pop(0)
                n = len(plist)
                at_sb = spool.tile([n * G, 128], f16, tag="atsb")
                nc.scalar.copy(at_sb[:], at_ps[:])
                s_copy.append((tick, plist, at_sb))

            def do_flat(tick):
                t0, plist, at_sb = s_copy.pop(0)
                n = len(plist)
                aflat = spool.tile([1, n * WPS], f16, tag="aflat")
                nc.gpsimd.dma_start(aflat[:], at_sb[:])
                s_flat.append((tick, plist, aflat))

            def do_bc(tick):
                t0, plist, aflat = s_flat.pop(0)
                bcs = []
                for i in range(len(plist)):
                    bc_ps = bc_pool.tile([128, WPS], f32, tag="bc")
                    for h in range(2):
                        nc.tensor.matmul(
                            bc_ps[:, h * 512 : (h + 1) * 512],
                            lhsT=ones_sb[:],
                            rhs=aflat[:, i * WPS + h * 512 : i * WPS + (h + 1) * 512],
                            start=True,
                            stop=True,
                        )
                    bcs.append(bc_ps)
                s_bc.append((tick, plist, bcs))

            def do_bccopy(tick):
                t0, plist, bcs = s_bc.pop(0)
                for (ss, xv), bc_ps in zip(plist, bcs):
                    bc_sb = bcs_pool.tile([128, WPS], f16, tag="bcsb")
                    nc.scalar.copy(bc_sb[:], bc_ps[:])
                    flights.append((ss, xv, bc_sb))

            def run_stages(tick, min_lag_bc=2):
                # oldest (final-most) stages first so each engine's stream
                # sees oldest-dependency instructions first
                while len(flights) > 1:
                    emit_final(*flights.pop(0))
                if s_bc:
                    do_bccopy(tick)
                if s_flat and s_flat[0][0] <= tick - min_lag_bc:
                    do_bc(tick)
                if s_copy:
                    do_flat(tick)
                if s_tran:
                    do_copy(tick)
                if s_alpha:
                    do_tran(tick)
                if ready and ready[0][0] < tick:
                    do_alpha(tick)

            s = 0
            rem_done = not REM
            for gsz in GROUPS:
                gt = xpool.tile([128, gsz * WPS], f16, tag="x")
                nc.sync.dma_start(gt[:], xd[:, s * WPS : (s + gsz) * WPS])
                for ls in range(gsz):
                    tick = s + ls
                    run_stages(tick)
                    xv = gt[:, ls * WPS : (ls + 1) * WPS]
                    if not pair:
                        pt_cur[0] = pps_pool.tile(
                            [128, 2 * L * G], f32, name="pt", tag="pt"
                        )
                    i = len(pair)
                    stage_P(xv, pt_cur[0][:, i * L * G : (i + 1) * L * G])
                    pair.append((tick, xv))
                    if len(pair) == 2:
                        ready.append((tick, list(pair), pt_cur[0]))
                        pair.clear()
                s += gsz
                if not rem_done and s >= 24:
                    # Mid-stream: independent work, fills scheduling slack
                    # without delaying the first loads or the kernel tail.
                    block_rem()
                    rem_done = True
            # odd NSUP: flush the unpaired last supertile
            if pair:
                ready.append((NSUP, list(pair), pt_cur[0]))
                pair.clear()
            # drain
            tick = NSUP + 1
            while ready or s_alpha or s_tran or s_copy or s_flat or s_bc or flights:
                while len(flights) > 0:
                    emit_final(*flights.pop(0))
                if s_bc:
                    do_bccopy(tick)
                elif s_flat:
                    do_bc(tick)
                elif s_copy:
                    do_flat(tick)
                elif s_tran:
                    do_copy(tick)
                elif s_alpha:
                    do_tran(tick)
                elif ready:
                    do_alpha(tick)
                tick += 1

    nc.compile()
    return nc


def kernel(inputs, kernels, biases):
    global LAST_RESULTS
    import os

    if os.environ.get("BASS_TRACE"):
        # run_bass_kernel_spmd's trace path hard-imports antenv.axon_hooks,
        # which not every image ships; fall back to no-trace instead of
        # crashing when it is absent.
        try:
            import antenv.axon_hooks  # noqa: F401
        except ImportError:
            os.environ["BASS_NEVER_TRACE"] = "1"

    from concourse.bass_utils import run_bass_kernel_spmd

    x = np.ascontiguousarray(np.asarray(inputs), dtype=np.float32)
    assert x.shape == (B, D), x.shape
    kern = np.asarray(kernels, dtype=np.float32).reshape(L, D)
    bias = np.asarray(biases, dtype=np.float32).reshape(L, D)

    W = np.ascontiguousarray(kern.T)  # [D, L]
    has_bias = bool(np.any(bias))
    cs = []
    beta = np.zeros(D, dtype=np.float32)
    for l in range(L):
        cs.append(float(np.dot(beta.astype(np.float64), kern[l].astype(np.float64))))
        beta = beta + bias[l]

    key = (has_bias, tuple(cs) if has_bias else None)
    nc = _CACHE.get(key)
    if nc is None:
        nc = _build(cs, has_bias)
        _CACHE[key] = nc

    in_maps = []
    for i in range(N_CORES):
        xs = x[i * ROWS : (i + 1) * ROWS]
        m = {
            "xd": np.ascontiguousarray(
                xs[: NSUP * SUP].astype(np.float16).T
            ),
            "w": W.astype(np.float16),
            "ident": np.eye(128, dtype=np.float16),
            "ones": np.ones((1, 128), dtype=np.float16),
        }
        if REM:
            m["xrem"] = xs[NSUP * SUP :].astype(np.float16)
        if has_bias:
            m["bb"] = beta.astype(np.float16).reshape(128, 1)
        in_maps.append(m)

    res = run_bass_kernel_spmd(nc, in_maps, core_ids=list(range(N_CORES)))
    LAST_RESULTS = res
    outs = []
    for i in range(N_CORES):
        o = res.results[i]["out"]  # [128, NSUP*1024] f16, d-major
        full = np.empty((ROWS, D), dtype=np.float32)
        full[: NSUP * SUP] = o.T.astype(np.float32)
        if REM:
            orem = res.results[i]["outr"].astype(np.float32)
            if has_bias:
                orem = orem + beta[None, :]
            full[NSUP * SUP :] = orem
        outs.append(full)
    return np.concatenate(outs, axis=0).astype(np.float32)


# revision 27
# speedup vs baseline: 1.0437x; 1.0437x over previous
"""CrossNet forward as a Trainium2 Bass/Tile kernel, data-parallel over 8 cores.

Math: the CrossNet layer stack
    x_{l+1} = x0 * (x_l . w_l) + b_l + x_l            (l = 0..3)
collapses in closed form.  Writing x_l = x0 * alpha_l[b] + beta_l[d]:
    p_l[b]     = sum_d x0[b,d] w_l[d]                 (4 projections of x0)
    alpha_0    = 1,   alpha_{l+1} = alpha_l * (1 + p_l) + c_l
    beta_{l+1} = beta_l + b_l,  c_l = beta_l . w_l    (host-computable scalars)
    out        = x0 * alpha_4[b] + beta_4[d]

Memory-bound: 16 MB fp16 in + 16 MB fp16 out per core.  Pure-DMA floor
measured at 87.8 us (16 DMA engines, ~433 GB/s/core with 8 cores active).

v8 design (d-major layout):
- Host packs x fp16 D-MAJOR: xd[d, r] = x[r, d], so projections are direct
  PE matmuls (lhsT = x chunk [128d,128b], rhs = W [128d,4]) with NO
  transposes and NO PSUM->SBUF staging copies.  The previous row-major
  design spent 1.29 us/supertile on TensorE (4 packed f32 transposes + 8
  projections) + 0.65 us on ACT copies, which exceeded the 1.21
  us/supertile DMA pace and made TensorE the critical path (measured
  99.5 us).  d-major cuts TensorE to ~0.9 us/supertile.
- alpha comes out row-indexed [128b, G]; the final multiply in d-major
  needs it broadcast along the free dim.  Chain per PAIR of supertiles:
  PE transpose a16 [128,16]->[16,128] PSUM, ACT copy to SBUF, SBUF->SBUF
  flatten DMA [16,128]->[1,2048] (APs linearize independently), then per
  supertile two K=1 ones-matmuls [1,512]->[128,512] rebroadcast alpha
  into PSUM, one ACT copy -> fp16 [128,1024], one DVE 2x_1P multiply.
- Stores alternate between the gpsimd SWDGE ring and the scalar HWDGE
  ring: one ring's per-transfer boundary overhead made DMA engines 0/15
  ~10% hotter than the rest and their queue drained ~6 us after all other
  engines finished (measured).  Head/tail stores are single-supertile to
  prime the store stream early and shorten the last final->store chains.
- Emission is software-pipelined with >= 1 supertile-tick of lag between
  dependent stages so no engine stream head-of-line blocks on a same-tick
  cross-engine dependency.
- 36-row remainder keeps the old row-major mini-chain (fp16 transpose via
  ident), injected mid-stream.  float32r transposes corrupt packed fp16
  on HW (measured rel err ~3.5) -- not used anywhere anymore.
"""

import numpy as np

B = 500_000
D = 128
L = 4
N_CORES = 8
ROWS = B // N_CORES          # 62500 rows per core
G = 8                        # 128-row chunks per supertile
SUP = 128 * G                # 1024 rows per supertile
NSUP = ROWS // SUP           # 61 full supertiles
REM = ROWS - NSUP * SUP      # 36 remainder rows
WPS = SUP                    # fp16 words per partition per supertile (d-major)

# Load group sizes: small groups first for fast pipeline ramp, then 1 MB
# transfers (8 KB per partition contiguous) for line-rate descriptors.
# (2-supertile groups were tried: compute stalls shrank but 4 KB
# descriptors inflated load DMA engine-time by 13% -- net wash.)
GROUPS = [1, 1, 2] + [4] * 13 + [2, 2, 1]
assert sum(GROUPS) == NSUP

_CACHE: dict = {}

# test.py can read run metadata (exec_time_ns etc.) from here after a call.
LAST_RESULTS = None


def _build(cs, has_bias):
    import concourse.tile as tile
    from concourse import bacc, mybir

    f32 = mybir.dt.float32
    f16 = mybir.dt.float16

    nc = bacc.Bacc(
        "TRN2",
        target_bir_lowering=False,
        debug=False,
        enable_asserts=False,
        num_devices=N_CORES,
    )
    # D-major main block: xd[d, s*1024 + r] = x(s*1024 + r, d)
    xd = nc.dram_tensor("xd", [128, NSUP * WPS], f16, kind="ExternalInput").ap()
    xrem = None
    if REM:
        xrem = nc.dram_tensor("xrem", [REM, D], f16, kind="ExternalInput").ap()
    w = nc.dram_tensor("w", [D, L], f16, kind="ExternalInput").ap()
    ident = nc.dram_tensor("ident", [128, 128], f16, kind="ExternalInput").ap()
    ones = nc.dram_tensor("ones", [1, 128], f16, kind="ExternalInput").ap()
    bb = None
    if has_bias:
        # d-major bias: beta[d] is a per-partition scalar
        bb = nc.dram_tensor("bb", [128, 1], f16, kind="ExternalInput").ap()
    # Output in the SAME d-major layout; host transposes back.
    out = nc.dram_tensor("out", [128, NSUP * WPS], f16, kind="ExternalOutput").ap()
    outr = None
    if REM:
        outr = nc.dram_tensor("outr", [REM, D], f16, kind="ExternalOutput").ap()

    with tile.TileContext(nc) as tc:
        with (
            tc.tile_pool(name="consts", bufs=1) as cpool,
            tc.tile_pool(name="xin", bufs=6) as xpool,
            tc.tile_pool(name="ptps", bufs=2, space="PSUM") as pps_pool,
            tc.tile_pool(name="atps", bufs=1, space="PSUM") as at_pool,
            tc.tile_pool(name="bcps", bufs=5, space="PSUM") as bc_pool,
            tc.tile_pool(name="small", bufs=16) as spool,
            tc.tile_pool(name="bcsb", bufs=4) as bcs_pool,
            tc.tile_pool(name="outp", bufs=8) as opool,
        ):
            # Constants ride the ACT HWDGE ring so xd loads own the SP ring.
            w_sb = cpool.tile([D, L], f16, tag="w")
            nc.scalar.dma_start(w_sb[:], w)
            ones_sb = cpool.tile([1, 128], f16, tag="ones")
            nc.scalar.dma_start(ones_sb[:], ones)
            ident_sb = cpool.tile([128, 128], f16, tag="ident")
            nc.scalar.dma_start(ident_sb[:], ident)
            bb_sb = None
            if has_bias:
                bb_sb = cpool.tile([128, 1], f16, tag="bb")
                nc.scalar.dma_start(bb_sb[:], bb)

            copy_fn = mybir.ActivationFunctionType.Copy

            def alpha_from_pt(pt_ap, p_cnt, g_cnt):
                # q = 1 + p on the ACT engine (reads PSUM, frees DVE cycles),
                # then product over the 4 layers; fp16 result.
                # pt_ap is an already-sliced AP of shape [p_cnt, L * g_cnt].
                q_sb = spool.tile([p_cnt, L * g_cnt], f32, tag="q")
                nc.scalar.activation(q_sb[:], pt_ap, copy_fn, bias=1.0)
                a16 = spool.tile([p_cnt, g_cnt], f16, tag="a16")
                if has_bias:
                    qv = q_sb[:].rearrange("p (g l) -> p g l", l=L)
                    a = spool.tile([p_cnt, g_cnt], f32, tag="a0")
                    # c_0 == 0 always (beta_0 = 0)
                    nc.vector.tensor_copy(a[:], qv[:, :, 0])
                    for l in range(1, L):
                        t = spool.tile([p_cnt, g_cnt], f32, tag=f"a{l}")
                        nc.vector.tensor_mul(t[:], a[:], qv[:, :, l])
                        if cs[l] != 0.0:
                            t2 = spool.tile([p_cnt, g_cnt], f32, tag=f"ac{l}")
                            nc.vector.tensor_scalar_add(t2[:], t[:], float(cs[l]))
                            t = t2
                        a = t
                    nc.vector.tensor_copy(a16[:], a[:])
                else:
                    # t[g, u] = q[g, 2u] * q[g, 2u+1], then a = t[:,0]*t[:,1]
                    qp = q_sb[:].rearrange("p (g u l) -> p g u l", u=2, l=2)
                    t = spool.tile([p_cnt, 2 * g_cnt], f32, tag="a1")
                    tv = t[:].rearrange("p (g u) -> p g u", u=2)
                    nc.vector.tensor_mul(tv, qp[:, :, :, 0], qp[:, :, :, 1])
                    nc.vector.tensor_mul(a16[:], tv[:, :, 0], tv[:, :, 1])
                return a16

            def stage_P(xv, pt_slice):
                # 8 projection matmuls straight off the d-major x tile.
                for g in range(G):
                    nc.tensor.matmul(
                        pt_slice[:, g * L : (g + 1) * L],
                        lhsT=xv[:, g * 128 : (g + 1) * 128],
                        rhs=w_sb[:],
                        start=True,
                        stop=True,
                    )

            NB = 2  # supertiles per store batch (512 KB stores)
            N_EARLY = 4
            pair_tile = [None]
            batch_state = [None]
            store_idx = [0]

            def emit_final(s, xv, bc_sb):
                # d-major final multiply: all operands [128, 1024] fp16
                # unit-stride SBUF -> DVE 2x_1P.
                if pair_tile[0] is None:
                    bsz = 1 if (s < N_EARLY or s >= NSUP - 4) else NB
                    batch_state[0] = [s, bsz]
                start_s, bsz = batch_state[0]
                half = s - start_s
                if half == 0:
                    pair_tile[0] = opool.tile(
                        [128, bsz * WPS], f16, name="opair", tag="o"
                    )
                out_sb = pair_tile[0]
                oslice = out_sb[:, half * WPS : (half + 1) * WPS]
                if has_bias:
                    t_sb = opool.tile([128, WPS], f16, tag="t")
                    nc.vector.tensor_mul(t_sb[:], xv, bc_sb[:])
                    nc.vector.tensor_add(
                        oslice.unsqueeze(1),
                        t_sb[:].unsqueeze(1),
                        bb_sb[:].to_broadcast([128, 1, WPS]),
                    )
                else:
                    nc.vector.tensor_mul(oslice, xv, bc_sb[:])
                if half == bsz - 1 or s == NSUP - 1:
                    n = (half + 1) * WPS
                    s0 = s - half
                    # Alternate stores across the gpsimd SWDGE ring and the
                    # scalar HWDGE ring (idle after consts).
                    store_idx[0] += 1
                    eng = nc.gpsimd if store_idx[0] % 2 == 0 else nc.scalar
                    eng.dma_start(
                        out[:, s0 * WPS : s0 * WPS + n], out_sb[:, :n]
                    )
                    pair_tile[0] = None

            def block_rem():
                # Row-major mini-chain for the 36 leftover rows.
                p_cnt = REM
                x_sb = spool.tile([p_cnt, D], f16, tag="xr")
                nc.scalar.dma_start(x_sb[:], xrem)
                xt_ps = at_pool.tile([128, p_cnt], f16, tag="at")
                xt_sb = spool.tile([128, p_cnt], f16, tag="xtsb")
                pt_ps = pps_pool.tile([p_cnt, L], f32, tag="pt")
                nc.tensor.transpose(xt_ps[:], x_sb[:], ident_sb[:p_cnt, :p_cnt])
                nc.scalar.copy(xt_sb[:], xt_ps[:])
                nc.tensor.matmul(
                    pt_ps[:], lhsT=xt_sb[:], rhs=w_sb[:], start=True, stop=True
                )
                a16 = alpha_from_pt(pt_ps[:], p_cnt, 1)
                out_sb = spool.tile([p_cnt, D], f16, tag="or")
                if has_bias:
                    t_sb = spool.tile([p_cnt, D], f16, tag="tr")
                    nc.vector.tensor_mul(
                        t_sb[:].rearrange("p (u d) -> p u d", u=1),
                        x_sb[:].rearrange("p (u d) -> p u d", u=1),
                        a16[:].to_broadcast([p_cnt, 1, D]),
                    )
                    # row-major: beta indexed along the free dim; build from
                    # the d-major bb via broadcast is not available here, so
                    # reuse the per-partition ident trick is overkill --
                    # instead add via a [1, D] view is not expressible; fall
                    # back to a second DRAM const would be needed.  Biases
                    # are zero for this problem; keep the mul-only path
                    # correct and add bias on the host for the remainder.
                    nc.vector.tensor_copy(out_sb[:], t_sb[:])
                else:
                    nc.vector.tensor_mul(
                        out_sb[:].rearrange("p (u d) -> p u d", u=1),
                        x_sb[:].rearrange("p (u d) -> p u d", u=1),
                        a16[:].to_broadcast([p_cnt, 1, D]),
                    )
                nc.gpsimd.dma_start(outr, out_sb[:])

            # Software-pipelined emission: every stage consumes work emitted
            # on an EARLIER tick so no engine stream stalls on a same-tick
            # cross-engine dependency.  Per pair of supertiles:
            #   t:   projections (Tensor; 2nd member at t)
            #   t+1: q 1+p (ACT) + alpha products (DVE)
            #   t+2: alphaT transpose [128,16]->[16,128] (Tensor)
            #   t+3: ACT copy -> SBUF fp16
            #   t+4: flatten DMA [16,128]->[1,2048] (gpsimd ring)
            #   t+6: 4x K=1 ones-matmul rebroadcast (Tensor; 2 ticks after
            #        the flatten to cover SWDGE latency)
            #   t+7: ACT copy PSUM->SBUF fp16 [128,1024] x2
            #   t+8: DVE finals + batched stores (via flights, lag >= 1)
            pair = []     # [(s, xv)] projected into pt_cur, awaiting pair
            pt_cur = [None]
            ready = []    # (tick, plist, pt) alpha pending
            s_alpha = []  # (tick, plist, a16) transpose pending
            s_tran = []   # (tick, plist, at_ps) copy pending
            s_copy = []   # (tick, plist, at_sb) flatten pending
            s_flat = []   # (tick, plist, aflat) bcast pending
            s_bc = []     # (tick, plist, [bc_ps x n]) bc-copy pending
            flights = []  # (s, xv, bc_sb) final pending

            def do_alpha(tick):
                t0, plist, pt = ready.pop(0)
                n = len(plist)
                a16 = alpha_from_pt(pt[:, : n * L * G], 128, n * G)
                s_alpha.append((tick, plist, a16))

            def do_tran(tick):
                t0, plist, a16 = s_alpha.pop(0)
                n = len(plist)
                at_ps = at_pool.tile([n * G, 128], f16, tag="at")
                nc.tensor.transpose(at_ps[:], a16[:], ident_sb[:])
                s_tran.append((tick, plist, at_ps))

            def do_copy(tick):
                t0, plist, at_ps = s_tran.pop(0)
                n = len(plist)
                at_sb = spool.tile([n * G, 128], f16, tag="atsb")
                nc.scalar.copy(at_sb[:], at_ps[:])
                s_copy.append((tick, plist, at_sb))

            def do_flat(tick):
                t0, plist, at_sb = s_copy.pop(0)
                n = len(plist)
                aflat = spool.tile([1, n * WPS], f16, tag="aflat")
                nc.gpsimd.dma_start(aflat[:], at_sb[:])
                s_flat.append((tick, plist, aflat))

            def do_bc(tick):
                t0, plist, aflat = s_flat.pop(0)
                bcs = []
                for i in range(len(plist)):
                    halves = []
                    for h in range(2):
                        bc_ps = bc_pool.tile([128, 512], f32, tag="bc")
                        nc.tensor.matmul(
                            bc_ps[:],
                            lhsT=ones_sb[:],
                            rhs=aflat[:, i * WPS + h * 512 : i * WPS + (h + 1) * 512],
                            start=True,
                            stop=True,
                        )
                        halves.append(bc_ps)
                    bcs.append(halves)
                s_bc.append((tick, plist, bcs))

            def do_bccopy(tick):
                t0, plist, bcs = s_bc.pop(0)
                for (ss, xv), halves in zip(plist, bcs):
                    bc_sb = bcs_pool.tile([128, WPS], f16, tag="bcsb")
                    for h in range(2):
                        nc.scalar.copy(bc_sb[:, h * 512 : (h + 1) * 512], halves[h][:])
                    flights.append((ss, xv, bc_sb))

            def run_stages(tick, min_lag_bc=2):
                # oldest (final-most) stages first so each engine's stream
                # sees oldest-dependency instructions first
                while len(flights) > 1:
                    emit_final(*flights.pop(0))
                if s_bc:
                    do_bccopy(tick)
                if s_flat and s_flat[0][0] <= tick - min_lag_bc:
                    do_bc(tick)
                if s_copy:
                    do_flat(tick)
                if s_tran:
                    do_copy(tick)
                if s_alpha:
                    do_tran(tick)
                if ready and ready[0][0] < tick:
                    do_alpha(tick)

            s = 0
            rem_done = not REM
            for gsz in GROUPS:
                gt = xpool.tile([128, gsz * WPS], f16, tag="x")
                nc.sync.dma_start(gt[:], xd[:, s * WPS : (s + gsz) * WPS])
                for ls in range(gsz):
                    tick = s + ls
                    run_stages(tick)
                    xv = gt[:, ls * WPS : (ls + 1) * WPS]
                    if not pair:
                        pt_cur[0] = pps_pool.tile(
                            [128, 2 * L * G], f32, name="pt", tag="pt"
                        )
                    i = len(pair)
                    stage_P(xv, pt_cur[0][:, i * L * G : (i + 1) * L * G])
                    pair.append((tick, xv))
                    if len(pair) == 2:
                        ready.append((tick, list(pair), pt_cur[0]))
                        pair.clear()
                s += gsz
                if not rem_done and s >= 24:
                    # Mid-stream: independent work, fills scheduling slack
                    # without delaying the first loads or the kernel tail.
                    block_rem()
                    rem_done = True
            # odd NSUP: flush the unpaired last supertile
            if pair:
                ready.append((NSUP, list(pair), pt_cur[0]))
                pair.clear()
            # drain
            tick = NSUP + 1
            while ready or s_alpha or s_tran or s_copy or s_flat or s_bc or flights:
                while len(flights) > 0:
                    emit_final(*flights.pop(0))
                if s_bc:
                    do_bccopy(tick)
                elif s_flat:
                    do_bc(tick)
                elif s_copy:
                    do_flat(tick)
                elif s_tran:
                    do_copy(tick)
                elif s_alpha:
                    do_tran(tick)
                elif ready:
                    do_alpha(tick)
                tick += 1

    nc.compile()
    return nc


def kernel(inputs, kernels, biases):
    global LAST_RESULTS
    import os

    if os.environ.get("BASS_TRACE"):
        # run_bass_kernel_spmd's trace path hard-imports antenv.axon_hooks,
        # which not every image ships; fall back to no-trace instead of
        # crashing when it is absent.
        try:
            import antenv.axon_hooks  # noqa: F401
        except ImportError:
            os.environ["BASS_NEVER_TRACE"] = "1"

    from concourse.bass_utils import run_bass_kernel_spmd

    x = np.ascontiguousarray(np.asarray(inputs), dtype=np.float32)
    assert x.shape == (B, D), x.shape
    kern = np.asarray(kernels, dtype=np.float32).reshape(L, D)
    bias = np.asarray(biases, dtype=np.float32).reshape(L, D)

    W = np.ascontiguousarray(kern.T)  # [D, L]
    has_bias = bool(np.any(bias))
    cs = []
    beta = np.zeros(D, dtype=np.float32)
    for l in range(L):
        cs.append(float(np.dot(beta.astype(np.float64), kern[l].astype(np.float64))))
        beta = beta + bias[l]

    key = (has_bias, tuple(cs) if has_bias else None)
    nc = _CACHE.get(key)
    if nc is None:
        nc = _build(cs, has_bias)
        _CACHE[key] = nc

    in_maps = []
    for i in range(N_CORES):
        xs = x[i * ROWS : (i + 1) * ROWS]
        m = {
            "xd": np.ascontiguousarray(
                xs[: NSUP * SUP].astype(np.float16).T
            ),
            "w": W.astype(np.float16),
            "ident": np.eye(128, dtype=np.float16),
            "ones": np.ones((1, 128), dtype=np.float16),
        }
        if REM:
            m["xrem"] = xs[NSUP * SUP :].astype(np.float16)
        if has_bias:
            m["bb"] = beta.astype(np.float16).reshape(128, 1)
        in_maps.append(m)

    res = run_bass_kernel_spmd(nc, in_maps, core_ids=list(range(N_CORES)))
    LAST_RESULTS = res
    outs = []
    for i in range(N_CORES):
        o = res.results[i]["out"]  # [128, NSUP*1024] f16, d-major
        full = np.empty((ROWS, D), dtype=np.float32)
        full[: NSUP * SUP] = o.T.astype(np.float32)
        if REM:
            orem = res.results[i]["outr"].astype(np.float32)
            if has_bias:
                orem = orem + beta[None, :]
            full[NSUP * SUP :] = orem
        outs.append(full)
    return np.concatenate(outs, axis=0).astype(np.float32)


# revision 28
# speedup vs baseline: 1.4784x; 1.4164x over previous
"""CrossNet forward as a Trainium2 Bass/Tile kernel, data-parallel over 8 cores.

Math: the CrossNet layer stack
    x_{l+1} = x0 * (x_l . w_l) + b_l + x_l            (l = 0..3)
collapses in closed form.  Writing x_l = x0 * alpha_l[b] + beta_l[d]:
    p_l[b]     = sum_d x0[b,d] w_l[d]                 (4 projections of x0)
    alpha_0    = 1,   alpha_{l+1} = alpha_l * (1 + p_l) + c_l
    beta_{l+1} = beta_l + b_l,  c_l = beta_l . w_l    (host-computable scalars)
    out        = x0 * alpha_4[b] + beta_4[d]

Memory-bound problem: 16 MB fp16 in + 16 MB fp16 out per core at the
~358 GB/s HBM-per-NC limit gives a ~90 us floor.  v2 design notes:

- Host packs x to fp16 pair-interleaved supertiles [128, (j d q)] where
  row = s*1024 + p*8 + 2j+q, stored PARTITION-MAJOR in DRAM
  ([128, NSUP*512] f32 words) so multi-supertile loads are one long
  contiguous run per partition (8 KB descriptors instead of 2 KB).
- Per supertile: 4 packed fp32-dtype PE transposes (bit-exact 16-bit-halves
  routing, 2 fp16 chunks per transpose), ACT copies PSUM->SBUF, 8 fp16
  [128d,128b]^T @ [128d,4] projection matmuls on strided fp16 views, tiny
  f32 DVE recurrence for alpha with the last op emitting fp16.
- The final multiply keeps the OUTPUT in the same packed (j d q) layout so
  every operand (x, alpha-pair, out) is fp16 with innermost step +1: the
  alpha AP is the 32-bit pair [a1|a0] re-read with a stride-0 middle dim.
  That qualifies for the DVE 2x_1P dual-pump mode (mixed f32 broadcast ran
  1x and made stores trail loads by ~37 us in v1).
- fp16 store, host upcasts/unpacks.  Loads on the SP HWDGE ring, stores on
  the GpSimd SWDGE ring, consts on the ACT HWDGE ring so the first x load
  issues immediately.  The 36-row remainder runs FIRST so it does not
  serialize the kernel tail.  Final/store stage is software-pipelined one
  supertile behind the projection stage.
- float32r was measured to CORRUPT packed fp16 patterns on HW (rel err
  ~3.5) -- transposes must stay plain float32.
"""

import numpy as np

B = 500_000
D = 128
L = 4
N_CORES = 8
ROWS = B // N_CORES          # 62500 rows per core
G = 8                        # 128-row chunks per supertile
SUP = 128 * G                # 1024 rows per supertile
NSUP = ROWS // SUP           # 61 full supertiles
REM = ROWS - NSUP * SUP      # 36 remainder rows
NPAIR = G // 2               # 4 packed pairs per supertile
WPS = G * D // 2             # 512 fp32 words per partition per supertile

# Load group sizes: small groups first for fast pipeline ramp, then 1 MB
# transfers (8 KB per partition contiguous) for line-rate descriptors.
# (2-supertile groups were tried: compute stalls shrank but 4 KB descriptors
# inflated load DMA engine-time by 13% — net wash.  4-sup groups it is.)
GROUPS = [1, 1, 2] + [4] * 13 + [2, 2, 1]
assert sum(GROUPS) == NSUP

TDT = "float32"

_CACHE: dict = {}

# test.py can read run metadata (exec_time_ns etc.) from here after a call.
LAST_RESULTS = None


def _build(cs, has_bias):
    import concourse.tile as tile
    from concourse import bacc, mybir

    f32 = mybir.dt.float32
    f16 = mybir.dt.float16
    tdt = getattr(mybir.dt, TDT)
    mult = mybir.AluOpType.mult
    add = mybir.AluOpType.add

    nc = bacc.Bacc(
        "TRN2",
        target_bir_lowering=False,
        debug=False,
        enable_asserts=False,
        num_devices=N_CORES,
    )
    # Partition-major packed input: word (p, s*512 + j*128 + d) holds the
    # fp16 pair [x(r1,d) | x(r0,d)] with r(q) = s*1024 + p*8 + 2j + q.
    xp = nc.dram_tensor("xp", [128, NSUP * WPS], tdt, kind="ExternalInput").ap()
    xrem = None
    if REM:
        xrem = nc.dram_tensor("xrem", [REM, D], f16, kind="ExternalInput").ap()
    w = nc.dram_tensor("w", [D, L], f16, kind="ExternalInput").ap()
    ident = nc.dram_tensor("ident", [128, 128], f16, kind="ExternalInput").ap()
    ident32 = nc.dram_tensor("ident32", [128, 128], tdt, kind="ExternalInput").ap()
    bb = None
    if has_bias:
        bb = nc.dram_tensor("bb", [128, 2 * D], f16, kind="ExternalInput").ap()
    # Output in the SAME packed (j d q) partition-major layout; host unpacks.
    out = nc.dram_tensor("out", [128, NSUP * G * D], f16, kind="ExternalOutput").ap()
    outr = None
    if REM:
        outr = nc.dram_tensor("outr", [REM, D], f16, kind="ExternalOutput").ap()

    with tile.TileContext(nc) as tc:
        with (
            tc.tile_pool(name="consts", bufs=1) as cpool,
            tc.tile_pool(name="xin", bufs=6) as xpool,
            tc.tile_pool(name="xt", bufs=8) as xtpool,
            tc.tile_pool(name="xtps", bufs=5, space="PSUM") as tps_pool,
            tc.tile_pool(name="ptps", bufs=3, space="PSUM") as pps_pool,
            tc.tile_pool(name="small", bufs=16) as spool,
            tc.tile_pool(name="outp", bufs=12) as opool,
        ):
            # Constants ride the ACT HWDGE ring so xp loads own the SP ring.
            # ident32 first: it gates the main pipeline's transposes.
            ident32_sb = cpool.tile([128, 128], tdt, tag="ident32")
            nc.scalar.dma_start(ident32_sb[:], ident32)
            w_sb = cpool.tile([D, L], f16, tag="w")
            nc.scalar.dma_start(w_sb[:], w)
            ident_sb = cpool.tile([128, 128], f16, tag="ident")
            nc.scalar.dma_start(ident_sb[:], ident)
            bb_sb = None
            if has_bias:
                bb_sb = cpool.tile([128, 2 * D], f16, tag="bb")
                nc.scalar.dma_start(bb_sb[:], bb)

            copy_fn = mybir.ActivationFunctionType.Copy

            def alpha_from_pt(pt_ap, p_cnt, g_cnt):
                # q = 1 + p on the ACT engine (reads PSUM, frees DVE cycles),
                # then product over the 4 layers; fp16 result.
                # pt_ap is an already-sliced AP of shape [p_cnt, L * g_cnt].
                q_sb = spool.tile([p_cnt, L * g_cnt], f32, tag="q")
                nc.scalar.activation(q_sb[:], pt_ap, copy_fn, bias=1.0)
                a16 = spool.tile([p_cnt, g_cnt], f16, tag="a16")
                if has_bias:
                    qv = q_sb[:].rearrange("p (g l) -> p g l", l=L)
                    a = spool.tile([p_cnt, g_cnt], f32, tag="a0")
                    # c_0 == 0 always (beta_0 = 0)
                    nc.vector.tensor_copy(a[:], qv[:, :, 0])
                    for l in range(1, L):
                        last = l == L - 1
                        t = spool.tile([p_cnt, g_cnt], f32 if not last else f32,
                                       tag=f"a{l}")
                        nc.vector.tensor_mul(t[:], a[:], qv[:, :, l])
                        if cs[l] != 0.0:
                            t2 = spool.tile([p_cnt, g_cnt], f32, tag=f"ac{l}")
                            nc.vector.tensor_scalar_add(t2[:], t[:], float(cs[l]))
                            t = t2
                        a = t
                    nc.vector.tensor_copy(a16[:], a[:])
                else:
                    # t[g, u] = q[g, 2u] * q[g, 2u+1], then a = t[:,0]*t[:,1]
                    qp = q_sb[:].rearrange("p (g u l) -> p g u l", u=2, l=2)
                    t = spool.tile([p_cnt, 2 * g_cnt], f32, tag="a1")
                    tv = t[:].rearrange("p (g u) -> p g u", u=2)
                    nc.vector.tensor_mul(tv, qp[:, :, :, 0], qp[:, :, :, 1])
                    nc.vector.tensor_mul(a16[:], tv[:, :, 0], tv[:, :, 1])
                return a16

            def stage_T(xp32):
                # 4 packed PE transposes (2 fp16 chunks per transpose) into
                # one PSUM bank.  Emitted the tick the supertile arrives.
                xt_ps = tps_pool.tile([128, NPAIR * D], tdt, tag="xtps")
                for j in range(NPAIR):
                    nc.tensor.transpose(
                        xt_ps[:, j * D : (j + 1) * D],
                        xp32[:, j * D : (j + 1) * D],
                        ident32_sb[:],
                    )
                return xt_ps

            def stage_C(xt_ps):
                # ACT copy PSUM->SBUF, emitted one tick after stage_T so the
                # ACT stream never waits on a same-tick transpose.
                xt_sb = xtpool.tile([128, NPAIR * D], tdt, tag="xt")
                nc.scalar.copy(xt_sb[:], xt_ps[:])
                return xt_sb

            def stage_P(xt_sb, pt_slice):
                # 8 projection matmuls, emitted two ticks after stage_T so
                # the Tensor stream never waits on a same-tick ACT copy.
                xt16 = xt_sb[:].bitcast(f16).rearrange("d (j b q) -> d j b q", b=D, q=2)
                for g in range(G):
                    j, qq = g // 2, g % 2
                    nc.tensor.matmul(
                        pt_slice[:, g * L : (g + 1) * L],
                        lhsT=xt16[:, j, :, qq],
                        rhs=w_sb[:],
                        start=True,
                        stop=True,
                    )

            NB = 4  # supertiles per store batch (1 MB stores)
            # First stores go out in 1-supertile batches to prime the store
            # stream early (first store was landing at ~24 us, leaving the
            # 16-22 us window with loads blocked on full buffers and no
            # store work for the DMA engines).
            NB_EARLY = 1
            N_EARLY = 4
            pair_tile = [None]
            batch_state = [None]
            store_idx = [0]

            def emit_final(s, xp32, a16):
                # Packed-layout multiply: every operand fp16 innermost step 1
                # -> DVE dual-pump.  alpha pair [a1|a0] re-read across d.
                # NB supertiles share one output tile so stores are 1 MB
                # (8 KB/partition descriptors, 1/NB the SWDGE descgen work).
                # batch state: [start_s, bsz] of the open store batch.
                # Singles at the head (prime the store stream early) and the
                # tail (shortest possible last final->store chains).
                if pair_tile[0] is None:
                    bsz = 1 if (s < N_EARLY or s >= NSUP - 4) else NB
                    batch_state[0] = [s, bsz]
                start_s, bsz = batch_state[0]
                half = s - start_s
                if half == 0:
                    pair_tile[0] = opool.tile(
                        [128, bsz * G * D], f16, name="opair", tag="o"
                    )
                out_sb = pair_tile[0]
                base = half * G * D
                ov4 = out_sb[:, base : base + G * D].rearrange(
                    "p (j d q) -> p j d q", d=D, q=2
                )
                xv4 = xp32.bitcast(f16).rearrange("p (j d q) -> p j d q", d=D, q=2)
                av4 = (
                    a16[:]
                    .rearrange("p (j q) -> p j q", q=2)
                    .unsqueeze(2)
                    .broadcast_to([128, NPAIR, D, 2])
                )
                if has_bias:
                    t_sb = opool.tile([128, G * D], f16, tag="t")
                    tv4 = t_sb[:].rearrange("p (j d q) -> p j d q", d=D, q=2)
                    bv4 = (
                        bb_sb[:]
                        .rearrange("p (d q) -> p d q", q=2)
                        .unsqueeze(1)
                        .broadcast_to([128, NPAIR, D, 2])
                    )
                    nc.vector.tensor_mul(tv4, xv4, av4)
                    nc.vector.tensor_add(ov4, tv4, bv4)
                else:
                    nc.vector.tensor_mul(ov4, xv4, av4)
                if half == bsz - 1 or s == NSUP - 1:
                    n = (half + 1) * G * D
                    s0 = s - half
                    # Alternate stores across the gpsimd SWDGE ring and the
                    # scalar HWDGE ring (idle after consts): two queues halve
                    # each ring's per-transfer boundary costs and the tail
                    # overhang on the last-descriptor engines.
                    store_idx[0] += 1
                    eng = nc.gpsimd if store_idx[0] % 2 == 0 else nc.scalar
                    eng.dma_start(
                        out[:, s0 * G * D : s0 * G * D + n], out_sb[:, :n]
                    )
                    pair_tile[0] = None

            def block_rem():
                p_cnt = REM
                x_sb = xpool.tile([p_cnt, D], f16, tag="xr")
                # scalar ring: keep the SP ring exclusively for x loads
                nc.scalar.dma_start(x_sb[:], xrem)
                xt_ps = tps_pool.tile([128, p_cnt], f16, tag="xtps")
                xt_sb = xtpool.tile([128, p_cnt], f16, tag="xtr")
                pt_ps = pps_pool.tile([p_cnt, L], f32, tag="pt")
                nc.tensor.transpose(xt_ps[:], x_sb[:], ident_sb[:p_cnt, :p_cnt])
                nc.scalar.copy(xt_sb[:], xt_ps[:])
                nc.tensor.matmul(
                    pt_ps[:], lhsT=xt_sb[:], rhs=w_sb[:], start=True, stop=True
                )
                a16 = alpha_from_pt(pt_ps[:], p_cnt, 1)
                out_sb = opool.tile([p_cnt, D], f16, tag="or")
                if has_bias:
                    t_sb = opool.tile([p_cnt, D], f16, tag="tr")
                    nc.vector.tensor_mul(
                        t_sb[:].rearrange("p (u d) -> p u d", u=1),
                        x_sb[:].rearrange("p (u d) -> p u d", u=1),
                        a16[:].to_broadcast([p_cnt, 1, D]),
                    )
                    bv = bb_sb[:p_cnt].rearrange("p (d q) -> p d q", q=2)[:, :, 0]
                    nc.vector.tensor_add(out_sb[:], t_sb[:], bv)
                else:
                    nc.vector.tensor_mul(
                        out_sb[:].rearrange("p (u d) -> p u d", u=1),
                        x_sb[:].rearrange("p (u d) -> p u d", u=1),
                        a16[:].to_broadcast([p_cnt, 1, D]),
                    )
                # SWDGE: a sync-ring store would head-of-line block the
                # group loads behind it in the HWDGE FIFO (measured +4.9 us).
                nc.gpsimd.dma_start(outr, out_sb[:])

            # Software-pipelined emission.  The engine sequencers execute
            # their streams IN ORDER, so an instruction whose cross-engine
            # dependency was emitted the same tick stalls its whole engine
            # stream (measured ~650 ns/supertile of Tensor idle waiting on
            # the same-tick ACT copy -> effective 1.4 us/supertile compute
            # rate that paced the whole kernel).  Emitting every stage with
            # >= 1 supertile of lag keeps all streams stall-free:
            #   tick s:  finals/stores (oldest), alpha(pair, >=1 tick old),
            #            P(s-2), C(s-1), T(s)
            # The alpha chain stays batched over PAIRS of supertiles (one pt
            # tile, one ACT +1, two DVE muls) to halve DVE small-op overhead.
            tq = []       # [(s, xv, xt_ps)] T emitted, C pending
            cq = []       # [(s, xv, xt_sb)] C emitted, P pending
            pair = []     # [(s, xv)] projected into pt_cur, awaiting pair
            pt_cur = [None]
            ready = []    # [(tick, [(s, xv) x2], pt)] alpha pending
            flights = []  # [(s, xp32, a16slice), ...] awaiting emit

            def do_P(tick):
                s2, xv2, xt_sb2 = cq.pop(0)
                if not pair:
                    pt_cur[0] = pps_pool.tile(
                        [128, 2 * L * G], f32, name="pt", tag="pt"
                    )
                i = len(pair)
                stage_P(xt_sb2, pt_cur[0][:, i * L * G : (i + 1) * L * G])
                pair.append((s2, xv2))
                if len(pair) == 2:
                    ready.append((tick, list(pair), pt_cur[0]))
                    pair.clear()

            def do_alpha():
                _, plist, pt = ready.pop(0)
                n = len(plist)
                a16 = alpha_from_pt(pt[:, : n * L * G], 128, n * G)
                for i, (ss, xv) in enumerate(plist):
                    flights.append((ss, xv, a16[:, i * G : (i + 1) * G]))

            s = 0
            rem_done = not REM
            for gsz in GROUPS:
                gt = xpool.tile([128, gsz * WPS], tdt, tag="x")
                nc.sync.dma_start(gt[:], xp[:, s * WPS : (s + gsz) * WPS])
                for ls in range(gsz):
                    tick = s + ls
                    while len(flights) > 1:
                        emit_final(*flights.pop(0))
                    if ready and ready[0][0] < tick:
                        do_alpha()
                    if len(cq) > 0 and len(tq) > 0:
                        do_P(tick)
                    if tq:
                        s1, xv1, xt_ps1 = tq.pop(0)
                        cq.append((s1, xv1, stage_C(xt_ps1)))
                    tq.append((tick, gt[:, ls * WPS : (ls + 1) * WPS],
                               stage_T(gt[:, ls * WPS : (ls + 1) * WPS])))
                s += gsz
                if not rem_done and s >= 24:
                    # Mid-stream: independent work, fills scheduling slack
                    # without delaying the first loads or the kernel tail.
                    block_rem()
                    rem_done = True
            # drain: keep ticking without new T stages
            tick = NSUP
            while tq or cq or pair or ready or flights:
                while len(flights) > (1 if (tq or cq or ready or pair) else 0):
                    emit_final(*flights.pop(0))
                if ready:
                    do_alpha()
                    continue
                if cq:
                    do_P(tick)
                    tick += 1
                    continue
                if tq:
                    s1, xv1, xt_ps1 = tq.pop(0)
                    cq.append((s1, xv1, stage_C(xt_ps1)))
                    continue
                if pair:
                    # odd NSUP: the last supertile has no pair partner
                    ready.append((tick, list(pair), pt_cur[0]))
                    pair.clear()

    nc.compile()
    return nc


def _pack_shard(xs):
    # xs: [ROWS, D] float32 -> fp16 packed partition-major [128, NSUP*512] f32
    # words; word (p, s, j, d) = [x(s*1024+p*8+2j+1, d) | x(s*1024+p*8+2j, d)].
    x16 = xs[: NSUP * SUP].astype(np.float16).reshape(NSUP, 128, NPAIR, 2, D)
    # -> (p, s, j, d, q)
    pk = np.ascontiguousarray(x16.transpose(1, 0, 2, 4, 3)).reshape(128, -1)
    return pk.view(np.float32)


def _unpack_out(o, orem):
    # o: [128, NSUP*1024] f16 packed (s, j, d, q) per partition -> [ROWS, D] f32
    o5 = o.reshape(128, NSUP, NPAIR, D, 2).transpose(1, 0, 2, 4, 3)
    main = np.ascontiguousarray(o5).reshape(NSUP * SUP, D)
    full = np.empty((ROWS, D), dtype=np.float32)
    full[: NSUP * SUP] = main
    if REM:
        full[NSUP * SUP :] = orem
    return full


def kernel(inputs, kernels, biases):
    global LAST_RESULTS
    import os

    if os.environ.get("BASS_TRACE"):
        # run_bass_kernel_spmd's trace path hard-imports antenv.axon_hooks,
        # which not every image ships; fall back to no-trace instead of
        # crashing when it is absent.
        try:
            import antenv.axon_hooks  # noqa: F401
        except ImportError:
            os.environ["BASS_NEVER_TRACE"] = "1"

    from concourse.bass_utils import run_bass_kernel_spmd

    x = np.ascontiguousarray(np.asarray(inputs), dtype=np.float32)
    assert x.shape == (B, D), x.shape
    kern = np.asarray(kernels, dtype=np.float32).reshape(L, D)
    bias = np.asarray(biases, dtype=np.float32).reshape(L, D)

    W = np.ascontiguousarray(kern.T)  # [D, L]
    has_bias = bool(np.any(bias))
    cs = []
    beta = np.zeros(D, dtype=np.float32)
    for l in range(L):
        cs.append(float(np.dot(beta.astype(np.float64), kern[l].astype(np.float64))))
        beta = beta + bias[l]

    key = (has_bias, tuple(cs) if has_bias else None)
    nc = _CACHE.get(key)
    if nc is None:
        nc = _build(cs, has_bias)
        _CACHE[key] = nc

    in_maps = []
    for i in range(N_CORES):
        xs = x[i * ROWS : (i + 1) * ROWS]
        m = {
            "xp": _pack_shard(xs),
            "w": W.astype(np.float16),
            "ident": np.eye(128, dtype=np.float16),
            "ident32": np.eye(128, dtype=np.float32),
        }
        if REM:
            m["xrem"] = xs[NSUP * SUP :].astype(np.float16)
        if has_bias:
            # bb[p, (d q)] = beta[d] for both q halves.
            bb16 = np.broadcast_to(
                beta.astype(np.float16)[None, :, None], (128, D, 2)
            )
            m["bb"] = np.ascontiguousarray(bb16).reshape(128, 2 * D)
        in_maps.append(m)

    res = run_bass_kernel_spmd(nc, in_maps, core_ids=list(range(N_CORES)))
    LAST_RESULTS = res
    outs = []
    for i in range(N_CORES):
        o = res.results[i]["out"]
        orem = res.results[i]["outr"] if REM else None
        outs.append(_unpack_out(o, orem))
    return np.concatenate(outs, axis=0).astype(np.float32)



# revision 29
# speedup vs baseline: 1.4930x; 1.0099x over previous
"""CrossNet forward as a Trainium2 Bass/Tile kernel, data-parallel over 8 cores.

Math: the CrossNet layer stack
    x_{l+1} = x0 * (x_l . w_l) + b_l + x_l            (l = 0..3)
collapses in closed form.  Writing x_l = x0 * alpha_l[b] + beta_l[d]:
    p_l[b]     = sum_d x0[b,d] w_l[d]                 (4 projections of x0)
    alpha_0    = 1,   alpha_{l+1} = alpha_l * (1 + p_l) + c_l
    beta_{l+1} = beta_l + b_l,  c_l = beta_l . w_l    (host-computable scalars)
    out        = x0 * alpha_4[b] + beta_4[d]

Memory-bound problem: 16 MB fp16 in + 16 MB fp16 out per core at the
~358 GB/s HBM-per-NC limit gives a ~90 us floor.  v2 design notes:

- Host packs x to fp16 pair-interleaved supertiles [128, (j d q)] where
  row = s*1024 + p*8 + 2j+q, stored PARTITION-MAJOR in DRAM
  ([128, NSUP*512] f32 words) so multi-supertile loads are one long
  contiguous run per partition (8 KB descriptors instead of 2 KB).
- Per supertile: 4 packed fp32-dtype PE transposes (bit-exact 16-bit-halves
  routing, 2 fp16 chunks per transpose), ACT copies PSUM->SBUF, 8 fp16
  [128d,128b]^T @ [128d,4] projection matmuls on strided fp16 views, tiny
  f32 DVE recurrence for alpha with the last op emitting fp16.
- The final multiply keeps the OUTPUT in the same packed (j d q) layout so
  every operand (x, alpha-pair, out) is fp16 with innermost step +1: the
  alpha AP is the 32-bit pair [a1|a0] re-read with a stride-0 middle dim.
  That qualifies for the DVE 2x_1P dual-pump mode (mixed f32 broadcast ran
  1x and made stores trail loads by ~37 us in v1).
- fp16 store, host upcasts/unpacks.  Loads on the SP HWDGE ring, stores on
  the GpSimd SWDGE ring, consts on the ACT HWDGE ring so the first x load
  issues immediately.  The 36-row remainder runs FIRST so it does not
  serialize the kernel tail.  Final/store stage is software-pipelined one
  supertile behind the projection stage.
- float32r was measured to CORRUPT packed fp16 patterns on HW (rel err
  ~3.5) -- transposes must stay plain float32.
"""

import numpy as np

B = 500_000
D = 128
L = 4
N_CORES = 8
ROWS = B // N_CORES          # 62500 rows per core
G = 8                        # 128-row chunks per supertile
SUP = 128 * G                # 1024 rows per supertile
NSUP = ROWS // SUP           # 61 full supertiles
REM = ROWS - NSUP * SUP      # 36 remainder rows
NPAIR = G // 2               # 4 packed pairs per supertile
WPS = G * D // 2             # 512 fp32 words per partition per supertile

# Load group sizes: small groups first for fast pipeline ramp, then 1 MB
# transfers (8 KB per partition contiguous) for line-rate descriptors.
# (2-supertile groups were tried: compute stalls shrank but 4 KB descriptors
# inflated load DMA engine-time by 13% — net wash.  4-sup groups it is.)
GROUPS = [1, 1, 2] + [4] * 13 + [2, 2, 1]
assert sum(GROUPS) == NSUP

TDT = "float32"

_CACHE: dict = {}

# test.py can read run metadata (exec_time_ns etc.) from here after a call.
LAST_RESULTS = None


def _build(cs, has_bias):
    import concourse.tile as tile
    from concourse import bacc, mybir

    f32 = mybir.dt.float32
    f16 = mybir.dt.float16
    tdt = getattr(mybir.dt, TDT)
    mult = mybir.AluOpType.mult
    add = mybir.AluOpType.add

    nc = bacc.Bacc(
        "TRN2",
        target_bir_lowering=False,
        debug=False,
        enable_asserts=False,
        num_devices=N_CORES,
    )
    # Partition-major packed input: word (p, s*512 + j*128 + d) holds the
    # fp16 pair [x(r1,d) | x(r0,d)] with r(q) = s*1024 + p*8 + 2j + q.
    xp = nc.dram_tensor("xp", [128, NSUP * WPS], tdt, kind="ExternalInput").ap()
    xrem = None
    if REM:
        xrem = nc.dram_tensor("xrem", [REM, D], f16, kind="ExternalInput").ap()
    w = nc.dram_tensor("w", [D, L], f16, kind="ExternalInput").ap()
    ident = nc.dram_tensor("ident", [128, 128], f16, kind="ExternalInput").ap()
    ident32 = nc.dram_tensor("ident32", [128, 128], tdt, kind="ExternalInput").ap()
    bb = None
    if has_bias:
        bb = nc.dram_tensor("bb", [128, 2 * D], f16, kind="ExternalInput").ap()
    # Output in the SAME packed (j d q) partition-major layout; host unpacks.
    out = nc.dram_tensor("out", [128, NSUP * G * D], f16, kind="ExternalOutput").ap()
    outr = None
    if REM:
        outr = nc.dram_tensor("outr", [REM, D], f16, kind="ExternalOutput").ap()

    with tile.TileContext(nc) as tc:
        with (
            tc.tile_pool(name="consts", bufs=1) as cpool,
            tc.tile_pool(name="xin", bufs=6) as xpool,
            tc.tile_pool(name="xt", bufs=8) as xtpool,
            tc.tile_pool(name="xtps", bufs=5, space="PSUM") as tps_pool,
            tc.tile_pool(name="ptps", bufs=3, space="PSUM") as pps_pool,
            tc.tile_pool(name="small", bufs=16) as spool,
            tc.tile_pool(name="outp", bufs=12) as opool,
        ):
            # Constants ride the ACT HWDGE ring so xp loads own the SP ring.
            # ident32 first: it gates the main pipeline's transposes.
            ident32_sb = cpool.tile([128, 128], tdt, tag="ident32")
            nc.scalar.dma_start(ident32_sb[:], ident32)
            w_sb = cpool.tile([D, L], f16, tag="w")
            nc.scalar.dma_start(w_sb[:], w)
            ident_sb = cpool.tile([128, 128], f16, tag="ident")
            nc.scalar.dma_start(ident_sb[:], ident)
            bb_sb = None
            if has_bias:
                bb_sb = cpool.tile([128, 2 * D], f16, tag="bb")
                nc.scalar.dma_start(bb_sb[:], bb)

            copy_fn = mybir.ActivationFunctionType.Copy

            def alpha_from_pt(pt_ap, p_cnt, g_cnt):
                # q = 1 + p on the ACT engine (reads PSUM, frees DVE cycles),
                # then product over the 4 layers; fp16 result.
                # pt_ap is an already-sliced AP of shape [p_cnt, L * g_cnt].
                q_sb = spool.tile([p_cnt, L * g_cnt], f32, tag="q")
                nc.scalar.activation(q_sb[:], pt_ap, copy_fn, bias=1.0)
                a16 = spool.tile([p_cnt, g_cnt], f16, tag="a16")
                if has_bias:
                    qv = q_sb[:].rearrange("p (g l) -> p g l", l=L)
                    a = spool.tile([p_cnt, g_cnt], f32, tag="a0")
                    # c_0 == 0 always (beta_0 = 0)
                    nc.vector.tensor_copy(a[:], qv[:, :, 0])
                    for l in range(1, L):
                        last = l == L - 1
                        t = spool.tile([p_cnt, g_cnt], f32 if not last else f32,
                                       tag=f"a{l}")
                        nc.vector.tensor_mul(t[:], a[:], qv[:, :, l])
                        if cs[l] != 0.0:
                            t2 = spool.tile([p_cnt, g_cnt], f32, tag=f"ac{l}")
                            nc.vector.tensor_scalar_add(t2[:], t[:], float(cs[l]))
                            t = t2
                        a = t
                    nc.vector.tensor_copy(a16[:], a[:])
                else:
                    # t[g, u] = q[g, 2u] * q[g, 2u+1], then a = t[:,0]*t[:,1]
                    qp = q_sb[:].rearrange("p (g u l) -> p g u l", u=2, l=2)
                    t = spool.tile([p_cnt, 2 * g_cnt], f32, tag="a1")
                    tv = t[:].rearrange("p (g u) -> p g u", u=2)
                    nc.vector.tensor_mul(tv, qp[:, :, :, 0], qp[:, :, :, 1])
                    nc.vector.tensor_mul(a16[:], tv[:, :, 0], tv[:, :, 1])
                return a16

            def stage_T(xp32):
                # 4 packed PE transposes (2 fp16 chunks per transpose) into
                # one PSUM bank.  Emitted the tick the supertile arrives.
                xt_ps = tps_pool.tile([128, NPAIR * D], tdt, tag="xtps")
                for j in range(NPAIR):
                    nc.tensor.transpose(
                        xt_ps[:, j * D : (j + 1) * D],
                        xp32[:, j * D : (j + 1) * D],
                        ident32_sb[:],
                    )
                return xt_ps

            def stage_C(xt_ps):
                # ACT copy PSUM->SBUF, emitted one tick after stage_T so the
                # ACT stream never waits on a same-tick transpose.
                xt_sb = xtpool.tile([128, NPAIR * D], tdt, tag="xt")
                nc.scalar.copy(xt_sb[:], xt_ps[:])
                return xt_sb

            def stage_P(xt_sb, pt_slice):
                # 8 projection matmuls, emitted two ticks after stage_T so
                # the Tensor stream never waits on a same-tick ACT copy.
                xt16 = xt_sb[:].bitcast(f16).rearrange("d (j b q) -> d j b q", b=D, q=2)
                for g in range(G):
                    j, qq = g // 2, g % 2
                    nc.tensor.matmul(
                        pt_slice[:, g * L : (g + 1) * L],
                        lhsT=xt16[:, j, :, qq],
                        rhs=w_sb[:],
                        start=True,
                        stop=True,
                    )

            NB = 2  # supertiles per store batch (512 KB stores)
            # First stores go out in 1-supertile batches to prime the store
            # stream early (first store was landing at ~24 us, leaving the
            # 16-22 us window with loads blocked on full buffers and no
            # store work for the DMA engines).
            NB_EARLY = 1
            N_EARLY = 4
            pair_tile = [None]
            batch_state = [None]
            store_idx = [0]

            def emit_final(s, xp32, a16):
                # Packed-layout multiply: every operand fp16 innermost step 1
                # -> DVE dual-pump.  alpha pair [a1|a0] re-read across d.
                # NB supertiles share one output tile so stores are 1 MB
                # (8 KB/partition descriptors, 1/NB the SWDGE descgen work).
                # batch state: [start_s, bsz] of the open store batch.
                # Singles at the head (prime the store stream early) and the
                # tail (shortest possible last final->store chains).
                if pair_tile[0] is None:
                    bsz = 1 if (s < N_EARLY or s >= NSUP - 4) else NB
                    batch_state[0] = [s, bsz]
                start_s, bsz = batch_state[0]
                half = s - start_s
                if half == 0:
                    pair_tile[0] = opool.tile(
                        [128, bsz * G * D], f16, name="opair", tag="o"
                    )
                out_sb = pair_tile[0]
                base = half * G * D
                ov4 = out_sb[:, base : base + G * D].rearrange(
                    "p (j d q) -> p j d q", d=D, q=2
                )
                xv4 = xp32.bitcast(f16).rearrange("p (j d q) -> p j d q", d=D, q=2)
                av4 = (
                    a16[:]
                    .rearrange("p (j q) -> p j q", q=2)
                    .unsqueeze(2)
                    .broadcast_to([128, NPAIR, D, 2])
                )
                if has_bias:
                    t_sb = opool.tile([128, G * D], f16, tag="t")
                    tv4 = t_sb[:].rearrange("p (j d q) -> p j d q", d=D, q=2)
                    bv4 = (
                        bb_sb[:]
                        .rearrange("p (d q) -> p d q", q=2)
                        .unsqueeze(1)
                        .broadcast_to([128, NPAIR, D, 2])
                    )
                    nc.vector.tensor_mul(tv4, xv4, av4)
                    nc.vector.tensor_add(ov4, tv4, bv4)
                else:
                    nc.vector.tensor_mul(ov4, xv4, av4)
                if half == bsz - 1 or s == NSUP - 1:
                    n = (half + 1) * G * D
                    s0 = s - half
                    # Alternate stores across the gpsimd SWDGE ring and the
                    # scalar HWDGE ring (idle after consts): two queues halve
                    # each ring's per-transfer boundary costs and the tail
                    # overhang on the last-descriptor engines.
                    store_idx[0] += 1
                    eng = nc.gpsimd if store_idx[0] % 2 == 0 else nc.scalar
                    eng.dma_start(
                        out[:, s0 * G * D : s0 * G * D + n], out_sb[:, :n]
                    )
                    pair_tile[0] = None

            def block_rem():
                p_cnt = REM
                x_sb = xpool.tile([p_cnt, D], f16, tag="xr")
                # scalar ring: keep the SP ring exclusively for x loads
                nc.scalar.dma_start(x_sb[:], xrem)
                xt_ps = tps_pool.tile([128, p_cnt], f16, tag="xtps")
                xt_sb = xtpool.tile([128, p_cnt], f16, tag="xtr")
                pt_ps = pps_pool.tile([p_cnt, L], f32, tag="pt")
                nc.tensor.transpose(xt_ps[:], x_sb[:], ident_sb[:p_cnt, :p_cnt])
                nc.scalar.copy(xt_sb[:], xt_ps[:])
                nc.tensor.matmul(
                    pt_ps[:], lhsT=xt_sb[:], rhs=w_sb[:], start=True, stop=True
                )
                a16 = alpha_from_pt(pt_ps[:], p_cnt, 1)
                out_sb = opool.tile([p_cnt, D], f16, tag="or")
                if has_bias:
                    t_sb = opool.tile([p_cnt, D], f16, tag="tr")
                    nc.vector.tensor_mul(
                        t_sb[:].rearrange("p (u d) -> p u d", u=1),
                        x_sb[:].rearrange("p (u d) -> p u d", u=1),
                        a16[:].to_broadcast([p_cnt, 1, D]),
                    )
                    bv = bb_sb[:p_cnt].rearrange("p (d q) -> p d q", q=2)[:, :, 0]
                    nc.vector.tensor_add(out_sb[:], t_sb[:], bv)
                else:
                    nc.vector.tensor_mul(
                        out_sb[:].rearrange("p (u d) -> p u d", u=1),
                        x_sb[:].rearrange("p (u d) -> p u d", u=1),
                        a16[:].to_broadcast([p_cnt, 1, D]),
                    )
                # SWDGE: a sync-ring store would head-of-line block the
                # group loads behind it in the HWDGE FIFO (measured +4.9 us).
                nc.gpsimd.dma_start(outr, out_sb[:])

            # Software-pipelined emission.  The engine sequencers execute
            # their streams IN ORDER, so an instruction whose cross-engine
            # dependency was emitted the same tick stalls its whole engine
            # stream (measured ~650 ns/supertile of Tensor idle waiting on
            # the same-tick ACT copy -> effective 1.4 us/supertile compute
            # rate that paced the whole kernel).  Emitting every stage with
            # >= 1 supertile of lag keeps all streams stall-free:
            #   tick s:  finals/stores (oldest), alpha(pair, >=1 tick old),
            #            P(s-2), C(s-1), T(s)
            # The alpha chain stays batched over PAIRS of supertiles (one pt
            # tile, one ACT +1, two DVE muls) to halve DVE small-op overhead.
            tq = []       # [(s, xv, xt_ps)] T emitted, C pending
            cq = []       # [(s, xv, xt_sb)] C emitted, P pending
            pair = []     # [(s, xv)] projected into pt_cur, awaiting pair
            pt_cur = [None]
            ready = []    # [(tick, [(s, xv) x2], pt)] alpha pending
            flights = []  # [(s, xp32, a16slice), ...] awaiting emit

            def do_P(tick):
                s2, xv2, xt_sb2 = cq.pop(0)
                if not pair:
                    pt_cur[0] = pps_pool.tile(
                        [128, 2 * L * G], f32, name="pt", tag="pt"
                    )
                i = len(pair)
                stage_P(xt_sb2, pt_cur[0][:, i * L * G : (i + 1) * L * G])
                pair.append((s2, xv2))
                if len(pair) == 2:
                    ready.append((tick, list(pair), pt_cur[0]))
                    pair.clear()

            def do_alpha():
                _, plist, pt = ready.pop(0)
                n = len(plist)
                a16 = alpha_from_pt(pt[:, : n * L * G], 128, n * G)
                for i, (ss, xv) in enumerate(plist):
                    flights.append((ss, xv, a16[:, i * G : (i + 1) * G]))

            s = 0
            rem_done = not REM
            for gsz in GROUPS:
                gt = xpool.tile([128, gsz * WPS], tdt, tag="x")
                nc.sync.dma_start(gt[:], xp[:, s * WPS : (s + gsz) * WPS])
                for ls in range(gsz):
                    tick = s + ls
                    while len(flights) > 1:
                        emit_final(*flights.pop(0))
                    if ready and ready[0][0] < tick:
                        do_alpha()
                    if len(cq) > 0 and len(tq) > 0:
                        do_P(tick)
                    if tq:
                        s1, xv1, xt_ps1 = tq.pop(0)
                        cq.append((s1, xv1, stage_C(xt_ps1)))
                    tq.append((tick, gt[:, ls * WPS : (ls + 1) * WPS],
                               stage_T(gt[:, ls * WPS : (ls + 1) * WPS])))
                s += gsz
                if not rem_done and s >= 24:
                    # Mid-stream: independent work, fills scheduling slack
                    # without delaying the first loads or the kernel tail.
                    block_rem()
                    rem_done = True
            # drain: keep ticking without new T stages
            tick = NSUP
            while tq or cq or pair or ready or flights:
                while len(flights) > (1 if (tq or cq or ready or pair) else 0):
                    emit_final(*flights.pop(0))
                if ready:
                    do_alpha()
                    continue
                if cq:
                    do_P(tick)
                    tick += 1
                    continue
                if tq:
                    s1, xv1, xt_ps1 = tq.pop(0)
                    cq.append((s1, xv1, stage_C(xt_ps1)))
                    continue
                if pair:
                    # odd NSUP: the last supertile has no pair partner
                    ready.append((tick, list(pair), pt_cur[0]))
                    pair.clear()

    nc.compile()
    return nc


def _pack_shard(xs):
    # xs: [ROWS, D] float32 -> fp16 packed partition-major [128, NSUP*512] f32
    # words; word (p, s, j, d) = [x(s*1024+p*8+2j+1, d) | x(s*1024+p*8+2j, d)].
    x16 = xs[: NSUP * SUP].astype(np.float16).reshape(NSUP, 128, NPAIR, 2, D)
    # -> (p, s, j, d, q)
    pk = np.ascontiguousarray(x16.transpose(1, 0, 2, 4, 3)).reshape(128, -1)
    return pk.view(np.float32)


def _unpack_out(o, orem):
    # o: [128, NSUP*1024] f16 packed (s, j, d, q) per partition -> [ROWS, D] f32
    o5 = o.reshape(128, NSUP, NPAIR, D, 2).transpose(1, 0, 2, 4, 3)
    main = np.ascontiguousarray(o5).reshape(NSUP * SUP, D)
    full = np.empty((ROWS, D), dtype=np.float32)
    full[: NSUP * SUP] = main
    if REM:
        full[NSUP * SUP :] = orem
    return full


def kernel(inputs, kernels, biases):
    global LAST_RESULTS
    import os

    if os.environ.get("BASS_TRACE"):
        # run_bass_kernel_spmd's trace path hard-imports antenv.axon_hooks,
        # which not every image ships; fall back to no-trace instead of
        # crashing when it is absent.
        try:
            import antenv.axon_hooks  # noqa: F401
        except ImportError:
            os.environ["BASS_NEVER_TRACE"] = "1"

    from concourse.bass_utils import run_bass_kernel_spmd

    x = np.ascontiguousarray(np.asarray(inputs), dtype=np.float32)
    assert x.shape == (B, D), x.shape
    kern = np.asarray(kernels, dtype=np.float32).reshape(L, D)
    bias = np.asarray(biases, dtype=np.float32).reshape(L, D)

    W = np.ascontiguousarray(kern.T)  # [D, L]
    has_bias = bool(np.any(bias))
    cs = []
    beta = np.zeros(D, dtype=np.float32)
    for l in range(L):
        cs.append(float(np.dot(beta.astype(np.float64), kern[l].astype(np.float64))))
        beta = beta + bias[l]

    key = (has_bias, tuple(cs) if has_bias else None)
    nc = _CACHE.get(key)
    if nc is None:
        nc = _build(cs, has_bias)
        _CACHE[key] = nc

    in_maps = []
    for i in range(N_CORES):
        xs = x[i * ROWS : (i + 1) * ROWS]
        m = {
            "xp": _pack_shard(xs),
            "w": W.astype(np.float16),
            "ident": np.eye(128, dtype=np.float16),
            "ident32": np.eye(128, dtype=np.float32),
        }
        if REM:
            m["xrem"] = xs[NSUP * SUP :].astype(np.float16)
        if has_bias:
            # bb[p, (d q)] = beta[d] for both q halves.
            bb16 = np.broadcast_to(
                beta.astype(np.float16)[None, :, None], (128, D, 2)
            )
            m["bb"] = np.ascontiguousarray(bb16).reshape(128, 2 * D)
        in_maps.append(m)

    res = run_bass_kernel_spmd(nc, in_maps, core_ids=list(range(N_CORES)))
    LAST_RESULTS = res
    outs = []
    for i in range(N_CORES):
        o = res.results[i]["out"]
        orem = res.results[i]["outr"] if REM else None
        outs.append(_unpack_out(o, orem))
    return np.concatenate(outs, axis=0).astype(np.float32)

